# revision 1
# baseline (speedup 1.0000x reference)
"""Cross-parent attention kernel for Trainium2 (8 NeuronCores, SPMD).

Problem (hardcoded from spec): B=4, T=64, Nf=Np=384, C=128, h=2, dh=64.
  q = q_in @ Wq.T + bq ; k/v likewise from kv_in
  per (b,t,head): attn = softmax(q k^T / sqrt(dh)) ; out_h = attn @ v
  out = concat_heads @ Wo.T + bo

Sharding: data-parallel over the 256 (b,t) pairs -> 32 pairs per core.

Per-core design notes:
  - x tiles PE-transposed (f32r) into ONE [128,384] PSUM tile per input,
    so each needs a single PSUM->SBUF copy (bf16).
  - q/k projections in bf16 (N=384, 1 cyc/row); their PSUM results are
    rounded to f32r so the scores matmuls keep ~13-bit mantissa.
  - v token-major bf16 with rank-1 "ones" columns in the same PSUM tile;
    attn @ [v|1] gives the softmax denominator row for free
    (no-max softmax: scores ~ N(0,1), fp32-safe).
  - exp(scoresT) on ACT straight out of PSUM.
  - denominator: DVE reciprocal of the PSUM row -> K=1 PE matmul
    broadcast -> DVE multiply to normalized bf16 (one-PSUM-input rule).
  - final projection split-K over heads (bf16) into one PSUM tile.
  - emission is software-pipelined across bt (stage lag) so every engine
    FIFO holds independent work from several iterations.
"""

import numpy as np

B, T, NF, C = 4, 64, 384, 128
H, DH = 2, 64
NCORES = 8
PAIRS = B * T  # 256
PER_CORE = PAIRS // NCORES  # 32
SCALE = 1.0 / np.sqrt(DH)  # 0.125

_CACHE = {}


def _build(has_bias, n_pairs=PER_CORE, repeat=1):
    import concourse.bacc as bacc
    import concourse.mybir as mybir
    from concourse.tile import TileContext

    F32 = mybir.dt.float32
    F32R = mybir.dt.float32r
    BF16 = mybir.dt.bfloat16
    AF = mybir.ActivationFunctionType
    MUL = mybir.AluOpType.mult

    nc = bacc.Bacc()

    xq_d = nc.declare_dram_parameter("xq", [PER_CORE, NF, C], F32R, isOutput=False)
    xkv_d = nc.declare_dram_parameter("xkv", [PER_CORE, NF, C], F32R, isOutput=False)
    wqt_d = nc.declare_dram_parameter("wqt", [C, C], BF16, isOutput=False)
    wkt_d = nc.declare_dram_parameter("wkt", [C, C], BF16, isOutput=False)
    wvt_d = nc.declare_dram_parameter("wvt", [C, C], BF16, isOutput=False)
    wot0_d = nc.declare_dram_parameter("wot0", [DH, C], BF16, isOutput=False)
    wot1_d = nc.declare_dram_parameter("wot1", [DH, C], BF16, isOutput=False)
    ident_d = nc.declare_dram_parameter("ident", [C, C], F32R, isOutput=False)
    ones64_d = nc.declare_dram_parameter("ones64", [1, DH], F32R, isOutput=False)
    ones1_d = nc.declare_dram_parameter("ones1", [1, C], BF16, isOutput=False)
    ones2_d = nc.declare_dram_parameter("ones2", [1, 2], BF16, isOutput=False)
    if has_bias:
        bq_d = nc.declare_dram_parameter("bqc", [C, 1], F32, isOutput=False)
        bk_d = nc.declare_dram_parameter("bkc", [C, 1], F32, isOutput=False)
        bop_d = nc.declare_dram_parameter("bop", [1, C], BF16, isOutput=False)
    out_d = nc.declare_dram_parameter("out", [PER_CORE, NF, C], F32, isOutput=True)

    with TileContext(nc) as tc:
        with (
            tc.tile_pool(name="static", bufs=1) as stat,
            tc.tile_pool(name="xin", bufs=3) as xin,
            tc.tile_pool(name="xt", bufs=3) as xtp,
            tc.tile_pool(name="qk", bufs=3) as qkp,
            tc.tile_pool(name="vexp", bufs=3) as vexp,
            tc.tile_pool(name="small", bufs=3) as smallp,
            tc.tile_pool(name="outp", bufs=3) as outp,
            tc.tile_pool(name="ps_tp", bufs=2, space="PSUM") as tps,
            tc.tile_pool(name="ps_pj", bufs=2, space="PSUM") as pjp,
            tc.tile_pool(name="ps_sc", bufs=2, space="PSUM") as scp,
            tc.tile_pool(name="ps_at", bufs=2, space="PSUM") as atp,
        ):
            # ---- static loads ----
            wqt = stat.tile([C, C], BF16, tag="wqt")
            wkt = stat.tile([C, C], BF16, tag="wkt")
            wvt = stat.tile([C, C], BF16, tag="wvt")
            wot0 = stat.tile([DH, C], BF16, tag="wot0")
            wot1 = stat.tile([DH, C], BF16, tag="wot1")
            ident = stat.tile([C, C], F32R, tag="ident")
            ones64 = stat.tile([DH + 1, DH], F32R, tag="ones64")
            ones1 = stat.tile([1, C], BF16, tag="ones1")
            ones2 = stat.tile([1, 2], BF16, tag="ones2")
            nc.sync.dma_start(out=wqt[:], in_=wqt_d[:])
            nc.sync.dma_start(out=wkt[:], in_=wkt_d[:])
            nc.sync.dma_start(out=wvt[:], in_=wvt_d[:])
            nc.sync.dma_start(out=wot0[:], in_=wot0_d[:])
            nc.sync.dma_start(out=wot1[:], in_=wot1_d[:])
            nc.sync.dma_start(out=ident[:], in_=ident_d[:])
            nc.sync.dma_start(out=ones64[DH : DH + 1, :], in_=ones64_d[:])
            nc.sync.dma_start(out=ones1[:], in_=ones1_d[:])
            nc.sync.dma_start(out=ones2[:], in_=ones2_d[:])
            if has_bias:
                bqc = stat.tile([C, 1], F32, tag="bqc")
                bkc = stat.tile([C, 1], F32, tag="bkc")
                bop = stat.tile([1, C], BF16, tag="bop")
                nc.sync.dma_start(out=bqc[:], in_=bq_d[:])
                nc.sync.dma_start(out=bkc[:], in_=bk_d[:])
                nc.sync.dma_start(out=bop[:], in_=bop_d[:])

            # per-iteration state handed between pipeline stages
            st = [dict() for _ in range(n_pairs)]

            def s0_load(n):
                s = st[n]
                s["xq"] = xin.tile([128, 3 * C], F32R, tag="xq", name=f"xq{n}")
                s["xkv"] = xin.tile([128, 3 * C], F32R, tag="xkv", name=f"xkv{n}")
                nc.sync.dma_start(
                    out=s["xq"][:].rearrange("p (a c) -> p a c", a=3),
                    in_=xq_d[n].rearrange("(a p) c -> p a c", p=128),
                )
                nc.sync.dma_start(
                    out=s["xkv"][:].rearrange("p (a c) -> p a c", a=3),
                    in_=xkv_d[n].rearrange("(a p) c -> p a c", p=128),
                )

            def s1_transpose(n):
                s = st[n]
                tq = tps.tile([C, NF], F32R, tag="tp", name=f"tq{n}")
                tkv = tps.tile([C, NF], F32R, tag="tp", name=f"tkv{n}")
                for a in range(3):
                    sl = slice(a * 128, (a + 1) * 128)
                    nc.tensor.transpose(tq[:, sl], s["xq"][:, sl], ident[:])
                    nc.tensor.transpose(tkv[:, sl], s["xkv"][:, sl], ident[:])
                s["xtq"] = xtp.tile([C, NF], BF16, tag="xtq", name=f"xtq{n}")
                s["xtkv"] = xtp.tile([C, NF], BF16, tag="xtkv", name=f"xtkv{n}")
                nc.vector.tensor_copy(s["xtq"][:], tq[:].bitcast(F32))
                nc.scalar.copy(s["xtkv"][:], tkv[:].bitcast(F32))

            def s2_proj(n):
                s = st[n]
                qps = pjp.tile([C, NF], F32, tag="pj", name=f"qps{n}")
                nc.tensor.matmul(qps[:], wqt[:], s["xtq"][:], start=True, stop=True)
                s["qt"] = qkp.tile([C, NF], F32R, tag="qt", name=f"qt{n}")
                if has_bias:
                    with nc.allow_low_precision(reason="f32r rounding of qT"):
                        nc.scalar.activation(
                            s["qt"][:], qps[:], AF.Identity, bias=bqc[:], scale=1.0
                        )
                else:
                    nc.vector.tensor_copy(s["qt"][:], qps[:])

                kps = pjp.tile([C, NF], F32, tag="pj", name=f"kps{n}")
                nc.tensor.matmul(kps[:], wkt[:], s["xtkv"][:], start=True, stop=True)
                s["kt"] = qkp.tile([C, NF], F32R, tag="kt", name=f"kt{n}")
                if has_bias:
                    with nc.allow_low_precision(reason="f32r rounding of kT"):
                        nc.scalar.activation(
                            s["kt"][:], kps[:], AF.Identity, bias=bkc[:], scale=1.0
                        )
                else:
                    nc.vector.tensor_copy(s["kt"][:], kps[:])

                # v: 3 chunk matmuls + rank-1 ones into one [128, 390] psum
                vps = pjp.tile([128, 3 * 130], F32, tag="pj", name=f"vps{n}")
                for a in range(3):
                    o = a * 130
                    nc.tensor.matmul(
                        vps[:, o : o + 128],
                        s["xtkv"][:, a * 128 : (a + 1) * 128],
                        wvt[:],
                        start=True,
                        stop=True,
                    )
                    nc.tensor.matmul(
                        vps[:, o + 128 : o + 130], ones1[:], ones2[:],
                        start=True, stop=True,
                    )
                s["v"] = vexp.tile([128, 3 * 130], BF16, tag="v", name=f"v{n}")
                # [p, a, h*65 + d] <- psum[p, a, h*64 + d]
                nc.vector.tensor_copy(
                    s["v"][:]
                    .rearrange("p (a x) -> p a x", a=3)[:, :, 0:130]
                    .rearrange("p a (h x) -> p a h x", h=2)[:, :, :, 0:DH],
                    vps[:]
                    .rearrange("p (a x) -> p a x", a=3)[:, :, 0:128]
                    .rearrange("p a (h d) -> p a h d", h=2),
                )
                nc.vector.tensor_copy(
                    s["v"][:]
                    .rearrange("p (a x) -> p a x", a=3)[:, :, 0:130]
                    .rearrange("p a (h x) -> p a h x", h=2)[:, :, :, DH : DH + 1],
                    vps[:]
                    .rearrange("p (a x) -> p a x", a=3)[:, :, 128:130]
                    .rearrange("p a (h o) -> p a h o", h=2),
                )

            def s3_attention(n):
                s = st[n]
                s["osb"] = outp.tile([DH, 2 * NF], BF16, tag="on", name=f"on{n}")
                atsb = [None, None]
                for h in range(H):
                    hs = h * DH
                    esb = vexp.tile([128, 3 * NF], BF16, tag="exp", name=f"e{n}_{h}")
                    for a in range(3):
                        scps = scp.tile([128, NF], F32, tag="sc", name=f"sc{n}_{h}{a}")
                        nc.tensor.matmul(
                            scps[:],
                            s["kt"][hs : hs + DH, a * 128 : (a + 1) * 128],
                            s["qt"][hs : hs + DH, :],
                            start=True,
                            stop=True,
                        )
                        nc.scalar.activation(
                            esb[:, a * NF : (a + 1) * NF], scps[:], AF.Exp, scale=1.0
                        )
                    at = atp.tile([DH + 1, NF], F32, tag="at", name=f"at{n}_{h}")
                    for a in range(3):
                        nc.tensor.matmul(
                            at[:],
                            s["v"][:, a * 130 + h * 65 : a * 130 + (h + 1) * 65],
                            esb[:, a * NF : (a + 1) * NF],
                            start=(a == 0),
                            stop=(a == 2),
                        )
                    # free the PSUM accumulator: copy (ACT) and recip (DVE)
                    # both read it immediately and in parallel
                    un = smallp.tile([DH, NF], F32, tag="un", name=f"un{n}_{h}")
                    nc.scalar.copy(un[:], at[0:DH, :])
                    rc = smallp.tile([DH + 1, NF], F32R, tag="rc", name=f"rc{n}_{h}")
                    with nc.allow_low_precision(reason="softmax denom recip"):
                        nc.vector.reciprocal(rc[DH : DH + 1, :], at[DH : DH + 1, :])
                    atsb[h] = (un, rc)
                for h in range(H):
                    un, rc = atsb[h]
                    bc = atp.tile([DH, NF], F32, tag="at", name=f"bc{n}_{h}")
                    nc.tensor.matmul(
                        bc[:], ones64[DH : DH + 1, :], rc[DH : DH + 1, :],
                        start=True, stop=True,
                    )
                    nc.vector.tensor_tensor(
                        s["osb"][:, h * NF : (h + 1) * NF], un[:], bc[:], op=MUL
                    )

            def s4_final(n):
                s = st[n]
                fps = tps.tile([128, 3 * C], F32, tag="tp", name=f"fps{n}")
                for a in range(3):
                    sl = slice(a * 128, (a + 1) * 128)
                    nc.tensor.matmul(
                        fps[:, sl],
                        s["osb"][:, 0 * NF + a * 128 : 0 * NF + (a + 1) * 128],
                        wot0[:],
                        start=True,
                        stop=False,
                    )
                    nc.tensor.matmul(
                        fps[:, sl],
                        s["osb"][:, 1 * NF + a * 128 : 1 * NF + (a + 1) * 128],
                        wot1[:],
                        start=False,
                        stop=not has_bias,
                    )
                    if has_bias:
                        nc.tensor.matmul(
                            fps[:, sl], ones1[:], bop[:], start=False, stop=True
                        )
                fout = outp.tile([128, 3 * C], F32, tag="fout", name=f"fo{n}")
                nc.vector.tensor_copy(fout[:], fps[:])
                nc.sync.dma_start(
                    out=out_d[n].rearrange("(a p) c -> p a c", p=128),
                    in_=fout[:].rearrange("p (a c) -> p a c", a=3),
                )
                st[n] = None  # release references

            # software-pipelined emission, oldest stage first
            stages = [s0_load, s1_transpose, s2_proj, s3_attention, s4_final]
            NS = len(stages)

            def emit_all():
                for i in range(n_pairs):
                    st[i] = dict()
                for step in range(n_pairs + NS - 1):
                    for k in range(NS - 1, -1, -1):
                        i = step - k
                        if 0 <= i < n_pairs:
                            stages[k](i)

            if repeat == 1:
                emit_all()
            else:
                with tc.For_i(0, repeat, 1):
                    emit_all()

    nc.finalize()
    return nc


def _get_nc(has_bias, n_pairs=PER_CORE, repeat=1):
    key = ("nc", has_bias, n_pairs, repeat)
    if key not in _CACHE:
        _CACHE[key] = _build(has_bias, n_pairs, repeat)
    return _CACHE[key]


def kernel(q_in, kv_in, Wq, bq, Wk, bk, Wv, bv, Wo, bo):
    import ml_dtypes
    from concourse.bass_utils import run_bass_kernel_spmd

    q_in = np.asarray(q_in, dtype=np.float32)
    kv_in = np.asarray(kv_in, dtype=np.float32)
    Wq = np.asarray(Wq, dtype=np.float32)
    Wk = np.asarray(Wk, dtype=np.float32)
    Wv = np.asarray(Wv, dtype=np.float32)
    Wo = np.asarray(Wo, dtype=np.float32)
    bq = np.asarray(bq, dtype=np.float32)
    bk = np.asarray(bk, dtype=np.float32)
    bv = np.asarray(bv, dtype=np.float32)
    bo = np.asarray(bo, dtype=np.float32)

    bf16 = ml_dtypes.bfloat16
    # fold 1/sqrt(dh) into Wq/bq; fold bv through softmax (rows sum to 1)
    # and Wo into the output bias: out = attn@(v0 + 1 bv) @ Wo.T + bo
    #                                  = attn@v0 @ Wo.T + 1 (Wo bv + bo).
    wqt = (np.ascontiguousarray(Wq.T) * np.float32(SCALE)).astype(bf16)
    wkt = np.ascontiguousarray(Wk.T).astype(bf16)
    wvt = np.ascontiguousarray(Wv.T).astype(bf16)
    wot = Wo.T  # [c, c']
    wot0 = np.ascontiguousarray(wot[0:DH, :]).astype(bf16)
    wot1 = np.ascontiguousarray(wot[DH:C, :]).astype(bf16)
    bqs = (bq * np.float32(SCALE)).reshape(C, 1)
    bks = bk.reshape(C, 1)
    bop = (Wo @ bv + bo).reshape(1, C)
    has_bias = bool(np.any(bqs) or np.any(bks) or np.any(bop))

    nc = _get_nc(has_bias)

    qf = q_in.reshape(PAIRS, NF, C)
    kf = kv_in.reshape(PAIRS, NF, C)

    common = {
        "wqt": wqt,
        "wkt": wkt,
        "wvt": wvt,
        "wot0": wot0,
        "wot1": wot1,
        "ident": np.eye(C, dtype=np.float32),
        "ones64": np.ones((1, DH), dtype=np.float32),
        "ones1": np.ones((1, C), dtype=bf16),
        "ones2": np.ones((1, 2), dtype=bf16),
    }
    if has_bias:
        common["bqc"] = bqs
        common["bkc"] = bks
        common["bop"] = bop.astype(bf16)

    in_maps = []
    for i in range(NCORES):
        m = dict(common)
        m["xq"] = np.ascontiguousarray(qf[i * PER_CORE : (i + 1) * PER_CORE])
        m["xkv"] = np.ascontiguousarray(kf[i * PER_CORE : (i + 1) * PER_CORE])
        in_maps.append(m)

    res = run_bass_kernel_spmd(nc, in_maps, list(range(NCORES)))
    out = np.concatenate([res.results[i]["out"] for i in range(NCORES)], axis=0)
    return out.reshape(B, T, NF, C)



# revision 4
# speedup vs baseline: 1.7901x; 1.7901x over previous
"""Cross-parent attention kernel for Trainium2 (8 NeuronCores, SPMD). v3

Problem (hardcoded from spec): B=4, T=64, Nf=Np=384, C=128, h=2, dh=64.
  q = q_in @ Wq.T ; k/v from kv_in ; per (b,t,head):
  attn = softmax(q k^T / sqrt(dh)) ; out = concat_heads(attn @ v) @ Wo.T

Sharding: data-parallel over the 256 (b,t) pairs -> 32 pairs per core.

v3 design (vs v1 baseline at ~358us HW):
  - inputs converted to bf16 on host and loaded PRE-TRANSPOSED via the
    X-bar DMA-transpose -> no PE transposes, no f32r round trip, half
    the input DMA bytes.  All 16 chunk tiles SBUF-resident.
  - scores via the M-matrix trick: M_h^T = Wk_h^T Wq_h * scale folded on
    host; A_h^T = M_h^T.T @ xkv^T with M stationary (K=128 full array),
    then scores_h[k,q] = A_h^T[:,k] . xq^T[:,q].  No q/k projections,
    no row-tiling needed, one [128,768] PSUM evac per pair.
  - exp batched over both heads: 3 ACT ops/pair of FD=768 (strided over
    a [128,1024] 2-bank PSUM score tile).
  - softmax denom via ones-columns appended to v (65-col stationary);
    reciprocal via the single-op DVE reciprocal_approx_fast (the v1
    iterative `reciprocal` is ~5x slower and was the main sim-vs-HW
    gap); broadcast to 64 partitions on the otherwise-idle GPSIMD
    (partition_broadcast); one DVE multiply normalizes both heads.
  - final projection with Wo^T stationary -> out [c', tok] accumulated
    over heads in one PSUM bank; stored as linear bf16 rows (perfect
    DMA descriptors); host transposes back and restores f32.
  - biases handled exactly on host: bk cancels in softmax; bq folds
    into xq via inv(Wq); bv/bo are a constant output row added on host.
"""

import numpy as np

B, T, NF, C = 4, 64, 384, 128
H, DH = 2, 64
NCORES = 8
PAIRS = B * T  # 256
PER_CORE = PAIRS // NCORES  # 32
SCALE = 1.0 / np.sqrt(DH)  # 0.125
CI = 2  # pairs per input dma-transpose chunk
CO = 4  # pairs per output dma chunk

_CACHE = {}


def _build(has_bias=False, n_pairs=PER_CORE, repeat=1):
    import concourse.bacc as bacc
    import concourse.mybir as mybir
    from concourse.tile import TileContext

    F32 = mybir.dt.float32
    BF16 = mybir.dt.bfloat16
    AF = mybir.ActivationFunctionType
    MUL = mybir.AluOpType.mult

    nc = bacc.Bacc()

    xq_d = nc.declare_dram_parameter("xq", [n_pairs * NF, C], BF16, isOutput=False)
    xkv_d = nc.declare_dram_parameter("xkv", [n_pairs * NF, C], BF16, isOutput=False)
    m0t_d = nc.declare_dram_parameter("m0t", [C, C], BF16, isOutput=False)
    m1t_d = nc.declare_dram_parameter("m1t", [C, C], BF16, isOutput=False)
    wvt_d = nc.declare_dram_parameter("wvt", [C, C], BF16, isOutput=False)
    wot0_d = nc.declare_dram_parameter("wot0", [DH, C], BF16, isOutput=False)
    wot1_d = nc.declare_dram_parameter("wot1", [DH, C], BF16, isOutput=False)
    out_d = nc.declare_dram_parameter("out", [C, n_pairs * NF], BF16, isOutput=True)

    NIC = n_pairs // CI

    with TileContext(nc) as tc:
        with (
            tc.tile_pool(name="static", bufs=1) as stat,
            tc.tile_pool(name="asb", bufs=3) as asbp,
            tc.tile_pool(name="vsb", bufs=4) as vsbp,
            tc.tile_pool(name="esb", bufs=3) as esbp,
            tc.tile_pool(name="rcp", bufs=2) as rcpp,
            tc.tile_pool(name="osb", bufs=3) as osbp,
            tc.tile_pool(name="fout", bufs=2) as foutp,
            tc.tile_pool(name="ps_a", bufs=1, space="PSUM") as apsp,
            tc.tile_pool(name="ps_v", bufs=1, space="PSUM") as vpsp,
            tc.tile_pool(name="ps_sc", bufs=1, space="PSUM") as scpp,
            tc.tile_pool(name="ps_at", bufs=1, space="PSUM") as atpp,
            tc.tile_pool(name="ps_f", bufs=1, space="PSUM") as fpsp,
        ):
            # ---- static weights ----
            m0t = stat.tile([C, C], BF16, tag="m0t")
            m1t = stat.tile([C, C], BF16, tag="m1t")
            wvt = stat.tile([C, C], BF16, tag="wvt")
            wot0 = stat.tile([DH, C], BF16, tag="wot0")
            wot1 = stat.tile([DH, C], BF16, tag="wot1")
            nc.sync.dma_start(out=m0t[:], in_=m0t_d[:])
            nc.sync.dma_start(out=m1t[:], in_=m1t_d[:])
            nc.sync.dma_start(out=wvt[:], in_=wvt_d[:])
            nc.sync.dma_start(out=wot0[:], in_=wot0_d[:])
            nc.sync.dma_start(out=wot1[:], in_=wot1_d[:])

            # resident transposed input chunks: [C, CI*NF] bf16 each
            xqt = [
                stat.tile([C, CI * NF], BF16, tag=f"xqt{ci}", name=f"xqt{ci}")
                for ci in range(NIC)
            ]
            xkt = [
                stat.tile([C, CI * NF], BF16, tag=f"xkt{ci}", name=f"xkt{ci}")
                for ci in range(NIC)
            ]

            st = [dict() for _ in range(n_pairs)]
            gst = {}

            def load_all():
                for ci in range(NIC):
                    r0 = ci * CI * NF
                    nc.sync.dma_start_transpose(
                        out=xqt[ci][:], in_=xq_d[r0 : r0 + CI * NF, :]
                    )
                    nc.sync.dma_start_transpose(
                        out=xkt[ci][:], in_=xkv_d[r0 : r0 + CI * NF, :]
                    )

            def s1_av(n):
                s = st[n]
                ci, off = divmod(n, CI)
                off *= NF
                xkv_sl = xkt[ci][:, off : off + NF]
                # A_h^T = M_h^T.T @ xkv^T, both heads into one 2-bank tile
                aps = apsp.tile([C, 1024], F32, tag="aps", name=f"aps{n}")
                nc.tensor.matmul(aps[:, 0:NF], m0t[:], xkv_sl, start=True, stop=True)
                nc.tensor.matmul(
                    aps[:, 512 : 512 + NF], m1t[:], xkv_sl, start=True, stop=True
                )
                s["ast"] = asbp.tile([C, 2 * NF], BF16, tag="ast", name=f"ast{n}")
                nc.vector.tensor_copy(
                    s["ast"][:].rearrange("p (h x) -> p h x", h=2),
                    aps[:].rearrange("p (h x) -> p h x", x=512)[:, :, 0:NF],
                )
                # v = xkv @ Wv^T, token-major; ones cols for the denominators
                vps = vpsp.tile([128, NF], F32, tag="vps", name=f"vps{n}")
                for a in range(3):
                    nc.tensor.matmul(
                        vps[:, a * 128 : (a + 1) * 128],
                        xkv_sl[:, a * 128 : (a + 1) * 128],
                        wvt[:],
                        start=True,
                        stop=True,
                    )
                s["v"] = vsbp.tile([128, 6 * 65], BF16, tag="v", name=f"v{n}")
                nc.scalar.copy(
                    s["v"][:].rearrange("p (s x) -> p s x", x=65)[:, :, 0:64],
                    vps[:].rearrange("p (s x) -> p s x", x=64),
                )
                nc.vector.memset(
                    s["v"][:].rearrange("p (s x) -> p s x", x=65)[:, :, 64:65], 1.0
                )

            def s2_scores(n):
                s = st[n]
                ci, off = divmod(n, CI)
                off *= NF
                xq_sl = xqt[ci][:, off : off + NF]
                s["esb"] = esbp.tile([128, 2 * 3 * NF], BF16, tag="esb", name=f"esb{n}")
                for a in range(3):
                    scps = scpp.tile([128, 1024], F32, tag="sc", name=f"sc{n}_{a}")
                    for h in range(H):
                        nc.tensor.matmul(
                            scps[:, h * 512 : h * 512 + NF],
                            s["ast"][:, h * NF + a * 128 : h * NF + (a + 1) * 128],
                            xq_sl,
                            start=True,
                            stop=True,
                        )
                    nc.scalar.activation(
                        s["esb"][:]
                        .rearrange("p (h x) -> p h x", h=2)[:, :, a * NF : (a + 1) * NF],
                        scps[:].rearrange("p (h x) -> p h x", x=512)[:, :, 0:NF],
                        AF.Exp,
                        scale=1.0,
                    )

            def s3_attn(n):
                s = st[n]
                at = atpp.tile([DH + 1, 1024], F32, tag="at", name=f"at{n}")
                for h in range(H):
                    for a in range(3):
                        nc.tensor.matmul(
                            at[:, h * 512 : h * 512 + NF],
                            s["v"][:, (2 * a + h) * 65 : (2 * a + h + 1) * 65],
                            s["esb"][:, (h * 3 + a) * NF : (h * 3 + a + 1) * NF],
                            start=(a == 0),
                            stop=(a == 2),
                        )
                # evacuate un+denom in one ACT op; frees the PSUM bank fast.
                un = osbp.tile([DH + 1, 2 * NF], F32, tag="un", name=f"un{n}")
                nc.scalar.copy(
                    un[:].rearrange("p (h x) -> p h x", h=2),
                    at[:].rearrange("p (h x) -> p h x", x=512)[:, :, 0:NF],
                )
                # denominator row lives at partition 64 where the fast recip
                # and the gpsimd broadcast both misbehave -> DMA-hop it to
                # partition 0 (engine-free), then recip @p0 and broadcast.
                dhop = rcpp.tile([1, 2 * NF], F32, tag="dhop", name=f"dhop{n}")
                nc.sync.dma_start(out=dhop[:], in_=un[DH : DH + 1, :])
                rc = rcpp.tile([1, 2 * NF], F32, tag="rc", name=f"rc{n}")
                nc.vector.reciprocal_approx_fast(out=rc[:], in_=dhop[:])
                rcb = rcpp.tile([DH, 2 * NF], F32, tag="rcb", name=f"rcb{n}")
                nc.gpsimd.partition_broadcast(rcb[:], rc[:])
                s["osb"] = osbp.tile([DH, 2 * NF], BF16, tag="osb", name=f"osb{n}")
                nc.vector.tensor_tensor(
                    s["osb"][:].rearrange("p (h x) -> p h x", h=2),
                    un[0:DH, :].rearrange("p (h x) -> p h x", h=2),
                    rcb[:].rearrange("p (h x) -> p h x", h=2),
                    op=MUL,
                )

            def s4_final(n):
                s = st[n]
                g, gi = divmod(n, CO)
                fps = fpsp.tile([C, NF], F32, tag="fps", name=f"fps{n}")
                nc.tensor.matmul(
                    fps[:], wot0[:], s["osb"][:, 0:NF], start=True, stop=False
                )
                nc.tensor.matmul(
                    fps[:], wot1[:], s["osb"][:, NF : 2 * NF], start=False, stop=True
                )
                if gi == 0:
                    gst[g] = foutp.tile([C, CO * NF], BF16, tag="fo", name=f"fo{g}")
                fout = gst[g]
                if n % 2 == 0:
                    nc.vector.tensor_copy(fout[:, gi * NF : (gi + 1) * NF], fps[:])
                else:
                    nc.scalar.copy(fout[:, gi * NF : (gi + 1) * NF], fps[:])
                if gi == CO - 1:
                    nc.sync.dma_start(
                        out=out_d[:, g * CO * NF : (g + 1) * CO * NF], in_=fout[:]
                    )
                    del gst[g]
                st[n] = None

            stages = [s1_av, s2_scores, s3_attn, s4_final]
            NS = len(stages)

            def emit_all():
                for i in range(n_pairs):
                    st[i] = dict()
                load_all()
                for step in range(n_pairs + NS - 1):
                    for k in range(NS - 1, -1, -1):
                        i = step - k
                        if 0 <= i < n_pairs:
                            stages[k](i)

            if repeat == 1:
                emit_all()
            else:
                with tc.For_i(0, repeat, 1):
                    emit_all()

    nc.finalize()
    return nc


def _get_nc(has_bias=False, n_pairs=PER_CORE, repeat=1):
    key = ("nc", n_pairs, repeat)
    if key not in _CACHE:
        _CACHE[key] = _build(has_bias, n_pairs, repeat)
    return _CACHE[key]


def kernel(q_in, kv_in, Wq, bq, Wk, bk, Wv, bv, Wo, bo):
    import ml_dtypes
    from concourse.bass_utils import run_bass_kernel_spmd

    bf16 = ml_dtypes.bfloat16

    q_in = np.asarray(q_in, dtype=np.float32)
    kv_in = np.asarray(kv_in, dtype=np.float32)
    Wq = np.asarray(Wq, dtype=np.float32)
    Wk = np.asarray(Wk, dtype=np.float32)
    Wv = np.asarray(Wv, dtype=np.float32)
    Wo = np.asarray(Wo, dtype=np.float32)
    bq = np.asarray(bq, dtype=np.float32)
    bk = np.asarray(bk, dtype=np.float32)
    bv = np.asarray(bv, dtype=np.float32)
    bo = np.asarray(bo, dtype=np.float32)

    # bq folds into xq exactly: (xq + bq @ inv(Wq).T) @ Wq.T = xq @ Wq.T + bq.
    # bk shifts every score of a query by the same constant -> softmax
    # invariant, dropped.  bv/bo: attn rows sum to 1, so they contribute the
    # constant row Wo @ bv + bo, added on the host at the end.
    if np.any(bq):
        q_in = q_in + (np.linalg.solve(Wq.T, bq)).astype(np.float32)

    # per-head M_h^T = Wk_h^T @ Wq_h, with the 1/sqrt(dh) scale folded in
    mts = []
    for h in range(H):
        Wq_h = Wq[h * DH : (h + 1) * DH, :]
        Wk_h = Wk[h * DH : (h + 1) * DH, :]
        mts.append((Wk_h.T @ Wq_h * np.float32(SCALE)).astype(bf16))

    wvt = np.ascontiguousarray(Wv.T).astype(bf16)
    wot = Wo.T  # [c, c']
    wot0 = np.ascontiguousarray(wot[0:DH, :]).astype(bf16)
    wot1 = np.ascontiguousarray(wot[DH:C, :]).astype(bf16)

    nc = _get_nc(False)

    qf = np.ascontiguousarray(q_in.reshape(PAIRS * NF, C)).astype(bf16)
    kf = np.ascontiguousarray(kv_in.reshape(PAIRS * NF, C)).astype(bf16)

    common = {
        "m0t": mts[0],
        "m1t": mts[1],
        "wvt": wvt,
        "wot0": wot0,
        "wot1": wot1,
    }
    in_maps = []
    rows = PER_CORE * NF
    for i in range(NCORES):
        m = dict(common)
        m["xq"] = qf[i * rows : (i + 1) * rows]
        m["xkv"] = kf[i * rows : (i + 1) * rows]
        in_maps.append(m)
    _CACHE["last_in_maps"] = in_maps

    res = run_bass_kernel_spmd(nc, in_maps, list(range(NCORES)))
    # each core's out is [C, PER_CORE*NF] bf16 (c-major) -> [tok, C] f32
    parts = []
    for i in range(NCORES):
        o = np.asarray(res.results[i]["out"], dtype=np.float32)  # [C, rows]
        parts.append(o.T)
    out = np.concatenate(parts, axis=0).reshape(B, T, NF, C)
    bias_row = Wo @ bv + bo
    if np.any(bias_row):
        out = out + bias_row
    return np.ascontiguousarray(out)


# revision 6
# speedup vs baseline: 1.8383x; 1.0269x over previous
"""Cross-parent attention kernel for Trainium2 (8 NeuronCores, SPMD). v3

Problem (hardcoded from spec): B=4, T=64, Nf=Np=384, C=128, h=2, dh=64.
  q = q_in @ Wq.T ; k/v from kv_in ; per (b,t,head):
  attn = softmax(q k^T / sqrt(dh)) ; out = concat_heads(attn @ v) @ Wo.T

Sharding: data-parallel over the 256 (b,t) pairs -> 32 pairs per core.

v3 design (vs v1 baseline at ~358us HW):
  - inputs converted to bf16 on host and loaded PRE-TRANSPOSED via the
    X-bar DMA-transpose -> no PE transposes, no f32r round trip, half
    the input DMA bytes.  All 16 chunk tiles SBUF-resident.
  - scores via the M-matrix trick: M_h^T = Wk_h^T Wq_h * scale folded on
    host; A_h^T = M_h^T.T @ xkv^T with M stationary (K=128 full array),
    then scores_h[k,q] = A_h^T[:,k] . xq^T[:,q].  No q/k projections,
    no row-tiling needed, one [128,768] PSUM evac per pair.
  - exp batched over both heads: 3 ACT ops/pair of FD=768 (strided over
    a [128,1024] 2-bank PSUM score tile).
  - softmax denom via ones-columns appended to v (65-col stationary);
    reciprocal via the single-op DVE reciprocal_approx_fast (the v1
    iterative `reciprocal` is ~5x slower and was the main sim-vs-HW
    gap); broadcast to 64 partitions on the otherwise-idle GPSIMD
    (partition_broadcast); one DVE multiply normalizes both heads.
  - final projection with Wo^T stationary -> out [c', tok] accumulated
    over heads in one PSUM bank; stored as linear bf16 rows (perfect
    DMA descriptors); host transposes back and restores f32.
  - biases handled exactly on host: bk cancels in softmax; bq folds
    into xq via inv(Wq); bv/bo are a constant output row added on host.
"""

import numpy as np

B, T, NF, C = 4, 64, 384, 128
H, DH = 2, 64
NCORES = 8
PAIRS = B * T  # 256
PER_CORE = PAIRS // NCORES  # 32
SCALE = 1.0 / np.sqrt(DH)  # 0.125
CI = 2  # pairs per input dma-transpose chunk
CO = 4  # pairs per output dma chunk

_CACHE = {}


def _build(has_bias=False, n_pairs=PER_CORE, repeat=1):
    import concourse.bacc as bacc
    import concourse.mybir as mybir
    from concourse.tile import TileContext

    F32 = mybir.dt.float32
    BF16 = mybir.dt.bfloat16
    AF = mybir.ActivationFunctionType
    MUL = mybir.AluOpType.mult

    nc = bacc.Bacc()

    xq_d = nc.declare_dram_parameter("xq", [n_pairs * NF, C], BF16, isOutput=False)
    xkv_d = nc.declare_dram_parameter("xkv", [n_pairs * NF, C], BF16, isOutput=False)
    m0t_d = nc.declare_dram_parameter("m0t", [C, C], BF16, isOutput=False)
    m1t_d = nc.declare_dram_parameter("m1t", [C, C], BF16, isOutput=False)
    wvt_d = nc.declare_dram_parameter("wvt", [C, C], BF16, isOutput=False)
    wot0_d = nc.declare_dram_parameter("wot0", [DH, C], BF16, isOutput=False)
    wot1_d = nc.declare_dram_parameter("wot1", [DH, C], BF16, isOutput=False)
    out_d = nc.declare_dram_parameter("out", [C, n_pairs * NF], BF16, isOutput=True)

    NIC = n_pairs // CI

    with TileContext(nc) as tc:
        with (
            tc.tile_pool(name="static", bufs=1) as stat,
            tc.tile_pool(name="asb", bufs=3) as asbp,
            tc.tile_pool(name="vsb", bufs=4) as vsbp,
            tc.tile_pool(name="esb", bufs=3) as esbp,
            tc.tile_pool(name="rcp", bufs=2) as rcpp,
            tc.tile_pool(name="osb", bufs=3) as osbp,
            tc.tile_pool(name="fout", bufs=2) as foutp,
            tc.tile_pool(name="ps_a", bufs=1, space="PSUM") as apsp,
            tc.tile_pool(name="ps_v", bufs=1, space="PSUM") as vpsp,
            tc.tile_pool(name="ps_sc", bufs=1, space="PSUM") as scpp,
            tc.tile_pool(name="ps_at", bufs=1, space="PSUM") as atpp,
            tc.tile_pool(name="ps_f", bufs=1, space="PSUM") as fpsp,
        ):
            # ---- static weights ----
            m0t = stat.tile([C, C], BF16, tag="m0t")
            m1t = stat.tile([C, C], BF16, tag="m1t")
            wvt = stat.tile([C, C], BF16, tag="wvt")
            wot0 = stat.tile([DH, C], BF16, tag="wot0")
            wot1 = stat.tile([DH, C], BF16, tag="wot1")
            nc.sync.dma_start(out=m0t[:], in_=m0t_d[:])
            nc.sync.dma_start(out=m1t[:], in_=m1t_d[:])
            nc.sync.dma_start(out=wvt[:], in_=wvt_d[:])
            nc.sync.dma_start(out=wot0[:], in_=wot0_d[:])
            nc.sync.dma_start(out=wot1[:], in_=wot1_d[:])

            # resident transposed input chunks: [C, CI*NF] bf16 each
            xqt = [
                stat.tile([C, CI * NF], BF16, tag=f"xqt{ci}", name=f"xqt{ci}")
                for ci in range(NIC)
            ]
            xkt = [
                stat.tile([C, CI * NF], BF16, tag=f"xkt{ci}", name=f"xkt{ci}")
                for ci in range(NIC)
            ]

            st = [dict() for _ in range(n_pairs)]
            gst = {}

            def load_all():
                for ci in range(NIC):
                    r0 = ci * CI * NF
                    nc.sync.dma_start_transpose(
                        out=xqt[ci][:], in_=xq_d[r0 : r0 + CI * NF, :]
                    )
                    nc.sync.dma_start_transpose(
                        out=xkt[ci][:], in_=xkv_d[r0 : r0 + CI * NF, :]
                    )

            def s1_av(n):
                s = st[n]
                ci, off = divmod(n, CI)
                off *= NF
                xkv_sl = xkt[ci][:, off : off + NF]
                # A_h^T = M_h^T.T @ xkv^T, both heads into one 2-bank tile
                aps = apsp.tile([C, 1024], F32, tag="aps", name=f"aps{n}")
                nc.tensor.matmul(aps[:, 0:NF], m0t[:], xkv_sl, start=True, stop=True)
                nc.tensor.matmul(
                    aps[:, 512 : 512 + NF], m1t[:], xkv_sl, start=True, stop=True
                )
                s["ast"] = asbp.tile([C, 2 * NF], BF16, tag="ast", name=f"ast{n}")
                nc.vector.tensor_copy(
                    s["ast"][:].rearrange("p (h x) -> p h x", h=2),
                    aps[:].rearrange("p (h x) -> p h x", x=512)[:, :, 0:NF],
                )
                # v = xkv @ Wv^T, token-major; ones cols for the denominators
                vps = vpsp.tile([128, NF], F32, tag="vps", name=f"vps{n}")
                for a in range(3):
                    nc.tensor.matmul(
                        vps[:, a * 128 : (a + 1) * 128],
                        xkv_sl[:, a * 128 : (a + 1) * 128],
                        wvt[:],
                        start=True,
                        stop=True,
                    )
                s["v"] = vsbp.tile([128, 6 * 65], BF16, tag="v", name=f"v{n}")
                nc.vector.tensor_copy(
                    s["v"][:].rearrange("p (s x) -> p s x", x=65)[:, :, 0:64],
                    vps[:].rearrange("p (s x) -> p s x", x=64),
                )
                nc.vector.memset(
                    s["v"][:].rearrange("p (s x) -> p s x", x=65)[:, :, 64:65], 1.0
                )

            def s2_scores(n):
                s = st[n]
                ci, off = divmod(n, CI)
                off *= NF
                xq_sl = xqt[ci][:, off : off + NF]
                s["esb"] = esbp.tile([128, 2 * 3 * NF], BF16, tag="esb", name=f"esb{n}")
                for a in range(3):
                    scps = scpp.tile([128, 1024], F32, tag="sc", name=f"sc{n}_{a}")
                    for h in range(H):
                        nc.tensor.matmul(
                            scps[:, h * 512 : h * 512 + NF],
                            s["ast"][:, h * NF + a * 128 : h * NF + (a + 1) * 128],
                            xq_sl,
                            start=True,
                            stop=True,
                        )
                    nc.scalar.activation(
                        s["esb"][:]
                        .rearrange("p (h x) -> p h x", h=2)[:, :, a * NF : (a + 1) * NF],
                        scps[:].rearrange("p (h x) -> p h x", x=512)[:, :, 0:NF],
                        AF.Exp,
                        scale=1.0,
                    )

            def s3_attn(n):
                s = st[n]
                at = atpp.tile([DH + 1, 1024], F32, tag="at", name=f"at{n}")
                for h in range(H):
                    for a in range(3):
                        nc.tensor.matmul(
                            at[:, h * 512 : h * 512 + NF],
                            s["v"][:, (2 * a + h) * 65 : (2 * a + h + 1) * 65],
                            s["esb"][:, (h * 3 + a) * NF : (h * 3 + a + 1) * NF],
                            start=(a == 0),
                            stop=(a == 2),
                        )
                # evacuate un+denom (one ACT + one DVE op, split by head);
                # frees the PSUM bank fast.
                un = osbp.tile([DH + 1, 2 * NF], F32, tag="un", name=f"un{n}")
                nc.scalar.copy(un[:, 0:NF], at[:, 0:NF])
                nc.vector.tensor_copy(un[:, NF : 2 * NF], at[:, 512 : 512 + NF])
                # denominator row lives at partition 64 where the fast recip
                # and the gpsimd broadcast both misbehave.  DMA-scatter it
                # across 64 partitions (engine-free), recip with 12 elem/lane,
                # DMA-gather back to a p0 row, then gpsimd-broadcast.
                dhop = rcpp.tile([DH, 12], F32, tag="dhop", name=f"dhop{n}")
                nc.sync.dma_start(out=dhop[:], in_=un[DH : DH + 1, :])
                rc2 = rcpp.tile([DH, 12], F32, tag="rc2", name=f"rc2{n}")
                nc.vector.reciprocal_approx_fast(out=rc2[:], in_=dhop[:])
                rc = rcpp.tile([1, 2 * NF], F32, tag="rc", name=f"rc{n}")
                nc.sync.dma_start(out=rc[:], in_=rc2[:])
                rcb = rcpp.tile([DH, 2 * NF], F32, tag="rcb", name=f"rcb{n}")
                nc.gpsimd.partition_broadcast(rcb[:], rc[:])
                s["osb"] = osbp.tile([DH, 2 * NF], BF16, tag="osb", name=f"osb{n}")
                nc.vector.tensor_tensor(
                    s["osb"][:].rearrange("p (h x) -> p h x", h=2),
                    un[0:DH, :].rearrange("p (h x) -> p h x", h=2),
                    rcb[:].rearrange("p (h x) -> p h x", h=2),
                    op=MUL,
                )

            def s4_final(n):
                s = st[n]
                g, gi = divmod(n, CO)
                fps = fpsp.tile([C, NF], F32, tag="fps", name=f"fps{n}")
                nc.tensor.matmul(
                    fps[:], wot0[:], s["osb"][:, 0:NF], start=True, stop=False
                )
                nc.tensor.matmul(
                    fps[:], wot1[:], s["osb"][:, NF : 2 * NF], start=False, stop=True
                )
                if gi == 0:
                    gst[g] = foutp.tile([C, CO * NF], BF16, tag="fo", name=f"fo{g}")
                fout = gst[g]
                if n % 2 == 0:
                    nc.vector.tensor_copy(fout[:, gi * NF : (gi + 1) * NF], fps[:])
                else:
                    nc.scalar.copy(fout[:, gi * NF : (gi + 1) * NF], fps[:])
                if gi == CO - 1:
                    nc.sync.dma_start(
                        out=out_d[:, g * CO * NF : (g + 1) * CO * NF], in_=fout[:]
                    )
                    del gst[g]
                st[n] = None

            stages = [s1_av, s2_scores, s3_attn, s4_final]
            NS = len(stages)

            def emit_all():
                for i in range(n_pairs):
                    st[i] = dict()
                load_all()
                for step in range(n_pairs + NS - 1):
                    for k in range(NS - 1, -1, -1):
                        i = step - k
                        if 0 <= i < n_pairs:
                            stages[k](i)

            if repeat == 1:
                emit_all()
            else:
                with tc.For_i(0, repeat, 1):
                    emit_all()

    nc.finalize()
    return nc


def _get_nc(has_bias=False, n_pairs=PER_CORE, repeat=1):
    key = ("nc", n_pairs, repeat)
    if key not in _CACHE:
        _CACHE[key] = _build(has_bias, n_pairs, repeat)
    return _CACHE[key]


def kernel(q_in, kv_in, Wq, bq, Wk, bk, Wv, bv, Wo, bo):
    import ml_dtypes
    from concourse.bass_utils import run_bass_kernel_spmd

    bf16 = ml_dtypes.bfloat16

    q_in = np.asarray(q_in, dtype=np.float32)
    kv_in = np.asarray(kv_in, dtype=np.float32)
    Wq = np.asarray(Wq, dtype=np.float32)
    Wk = np.asarray(Wk, dtype=np.float32)
    Wv = np.asarray(Wv, dtype=np.float32)
    Wo = np.asarray(Wo, dtype=np.float32)
    bq = np.asarray(bq, dtype=np.float32)
    bk = np.asarray(bk, dtype=np.float32)
    bv = np.asarray(bv, dtype=np.float32)
    bo = np.asarray(bo, dtype=np.float32)

    # bq folds into xq exactly: (xq + bq @ inv(Wq).T) @ Wq.T = xq @ Wq.T + bq.
    # bk shifts every score of a query by the same constant -> softmax
    # invariant, dropped.  bv/bo: attn rows sum to 1, so they contribute the
    # constant row Wo @ bv + bo, added on the host at the end.
    if np.any(bq):
        q_in = q_in + (np.linalg.solve(Wq.T, bq)).astype(np.float32)

    # per-head M_h^T = Wk_h^T @ Wq_h, with the 1/sqrt(dh) scale folded in
    mts = []
    for h in range(H):
        Wq_h = Wq[h * DH : (h + 1) * DH, :]
        Wk_h = Wk[h * DH : (h + 1) * DH, :]
        mts.append((Wk_h.T @ Wq_h * np.float32(SCALE)).astype(bf16))

    wvt = np.ascontiguousarray(Wv.T).astype(bf16)
    wot = Wo.T  # [c, c']
    wot0 = np.ascontiguousarray(wot[0:DH, :]).astype(bf16)
    wot1 = np.ascontiguousarray(wot[DH:C, :]).astype(bf16)

    nc = _get_nc(False)

    qf = np.ascontiguousarray(q_in.reshape(PAIRS * NF, C)).astype(bf16)
    kf = np.ascontiguousarray(kv_in.reshape(PAIRS * NF, C)).astype(bf16)

    common = {
        "m0t": mts[0],
        "m1t": mts[1],
        "wvt": wvt,
        "wot0": wot0,
        "wot1": wot1,
    }
    in_maps = []
    rows = PER_CORE * NF
    for i in range(NCORES):
        m = dict(common)
        m["xq"] = qf[i * rows : (i + 1) * rows]
        m["xkv"] = kf[i * rows : (i + 1) * rows]
        in_maps.append(m)
    _CACHE["last_in_maps"] = in_maps

    res = run_bass_kernel_spmd(nc, in_maps, list(range(NCORES)))
    # each core's out is [C, PER_CORE*NF] bf16 (c-major) -> [tok, C] f32
    parts = []
    for i in range(NCORES):
        o = np.asarray(res.results[i]["out"], dtype=np.float32)  # [C, rows]
        parts.append(o.T)
    out = np.concatenate(parts, axis=0).reshape(B, T, NF, C)
    bias_row = Wo @ bv + bo
    if np.any(bias_row):
        out = out + bias_row
    return np.ascontiguousarray(out)
